# revision 1
# baseline (speedup 1.0000x reference)
"""EnhancedCorrelationGNN Trainium2 kernel (8 NeuronCores, SPMD).

Strategy: destination-sorted edge processing with node-range output sharding.
 - Host (free): counting-sort edges by dst, partition nodes into 8 ranges of
   6272 (49 blocks x 128 nodes per core). Per block, edges are split by src
   half (dma_gather int16 index limit) and padded to 128-edge tiles with
   cross-core-uniform tile counts (one SPMD program).
 - Phase 1 (device): h = x @ W plus attention projections in ONE matmul per
   128-node tile (rhs = [W@a_dst | W | W@a_src] assembled on-chip), AllGather
   of the [h|attn_s] node table (768B rows). attn_d stays core-local.
 - Phase 2 (device): per 32-tile chunk, dma_gather of [h|as] rows by src and
   ad rows by dst; batched VectorE ops compute leaky-relu scores, ScalarE
   exp, messages; one-hot segment matrix via is_equal(dst_local, iota);
   per-tile TensorE matmul scatter-accumulates [msgs | p] into the block
   PSUM; per block normalize by 1/(sum p + 1e-10), add bias, DMA out.
 - No AllReduce: softmax denominators and sums stay core-local because
   output is sharded by destination node range.
"""
import sys

if "/opt/trn_rl_repo" not in sys.path:
    sys.path.insert(0, "/opt/trn_rl_repo")

import numpy as np

import concourse.bass as bass
import concourse.bacc as bacc
import concourse.mybir as mybir
import concourse.tile as tile
from concourse.bass_utils import run_bass_kernel_spmd

# ---------------------------------------------------------------- constants
N = 50000
E = 800000
IN_F = 128
H = 8
HD = 16
OUT_F = H * HD          # 128
ALPHA = 0.2
EPS = 1e-10

NCORES = 8
P = 128
NPC = 6272              # nodes per core = 49 * 128; 8*6272 = 50176 >= N
NPAD = NCORES * NPC     # 50176
NBLK = NPC // P         # 49
HALF = NPAD // 2        # 25088 (aligned to core boundary: cores 0-3 / 4-7)

ROW = 192               # table row floats: h(128) | as(8) | pad(56) -> 768B
AS_OFF = 128            # attn_s offset within row
ADROW = 64              # ad table row floats: ad(8) | pad(56) -> 256B
CHUNK_TILES = 16        # tiles per gather/DVE chunk
IDX_COLS = CHUNK_TILES * P // 16   # wrapped int16 idx columns per chunk
PAD_DSTL = 300.0        # one-hot miss sentinel (matches no iota value)

FP = mybir.dt.float32


# ---------------------------------------------------------------- planning
def _cdiv(a, b):
    return -(-a // b)


def _wrap_idx(idx_flat: np.ndarray) -> np.ndarray:
    """[n] -> [128, IDX_COLS] int16: idx j at [j%16, j//16], replicated x8."""
    n = idx_flat.shape[0]
    assert n % 16 == 0
    w = idx_flat.reshape(n // 16, 16).T.astype(np.int16)      # [16, n/16]
    w = np.tile(w, (8, 1))                                    # [128, n/16]
    out = np.zeros((P, IDX_COLS), dtype=np.int16)
    out[:, : w.shape[1]] = w
    return out


def plan_and_inputs(edge_index, edge_weight):
    """Host-side edge partitioning. Returns (plan, per_core_arrays).

    plan (core-independent, defines the SPMD program):
      KA, KB: [NBLK] tiles per (block, half)
      chunks: list of dicts(stream, g0, nt) over stream-major tile ids
      block_tiles: per block, list of (chunk_id, slot) in matmul order
      T, n_chunks
    per_core_arrays[c]:
      src_idx [n_chunks,128,IDX_COLS] i16 (half-relative)
      dst_idx [n_chunks,128,IDX_COLS] i16 (core-relative)
      dstl    [128, T] f32 (block-relative dst, PAD_DSTL for pad slots)
      ew      [128, T] f32
    """
    src = np.asarray(edge_index[0], dtype=np.int64)
    dst = np.asarray(edge_index[1], dtype=np.int64)
    ew = np.asarray(edge_weight, dtype=np.float32)

    order = np.argsort(dst, kind="stable")
    src_s, dst_s, ew_s = src[order], dst[order], ew[order]

    # per (core, block, half) edge index lists (into the sorted arrays)
    cnt = np.zeros((NCORES, NBLK, 2), dtype=np.int64)
    lists = [[[None, None] for _ in range(NBLK)] for _ in range(NCORES)]
    # block boundaries over sorted dst
    blk_starts = np.searchsorted(dst_s, np.arange(0, NPAD + 1, P))
    for c in range(NCORES):
        for b in range(NBLK):
            g = c * NBLK + b
            lo, hi = blk_starts[g], blk_starts[g + 1]
            s = src_s[lo:hi]
            mA = s < HALF
            idxs = np.arange(lo, hi)
            lists[c][b][0] = idxs[mA]
            lists[c][b][1] = idxs[~mA]
            cnt[c, b, 0] = mA.sum()
            cnt[c, b, 1] = (~mA).sum()

    KA = np.maximum(_cdiv(cnt[:, :, 0].max(axis=0), P), 1).astype(np.int64)
    KB = _cdiv(cnt[:, :, 1].max(axis=0), P).astype(np.int64)

    T_A = int(KA.sum())
    T_B = int(KB.sum())
    T = T_A + T_B
    cumKA = np.concatenate([[0], np.cumsum(KA)])
    cumKB = np.concatenate([[0], np.cumsum(KB)])

    # chunks: stream-major [0,T_A) then [T_A,T)
    chunks = []
    g = 0
    while g < T_A:
        nt = min(CHUNK_TILES, T_A - g)
        chunks.append(dict(stream=0, g0=g, nt=nt))
        g += nt
    while g < T:
        nt = min(CHUNK_TILES, T - g)
        chunks.append(dict(stream=1, g0=g, nt=nt))
        g += nt
    n_chunks = len(chunks)

    def tile_to_chunk(gidx):
        for ci, ch in enumerate(chunks):
            if ch["g0"] <= gidx < ch["g0"] + ch["nt"]:
                return ci, gidx - ch["g0"]
        raise AssertionError(gidx)

    # precompute chunk lookup as arrays for speed
    chunk_of = np.empty(T, dtype=np.int64)
    slot_of = np.empty(T, dtype=np.int64)
    for ci, ch in enumerate(chunks):
        chunk_of[ch["g0"]: ch["g0"] + ch["nt"]] = ci
        slot_of[ch["g0"]: ch["g0"] + ch["nt"]] = np.arange(ch["nt"])

    block_tiles = []
    for b in range(NBLK):
        tl = []
        for k in range(KA[b]):
            gidx = cumKA[b] + k
            tl.append((int(chunk_of[gidx]), int(slot_of[gidx])))
        for k in range(KB[b]):
            gidx = T_A + cumKB[b] + k
            tl.append((int(chunk_of[gidx]), int(slot_of[gidx])))
        block_tiles.append(tl)

    plan = dict(KA=KA, KB=KB, T=T, T_A=T_A, chunks=chunks,
                block_tiles=block_tiles, n_chunks=n_chunks)

    # ---------------- per-core slot arrays
    per_core = []
    for c in range(NCORES):
        src_rel = np.zeros((T, P), dtype=np.int16)
        dst_rel = np.zeros((T, P), dtype=np.int16)
        dstl = np.full((T, P), PAD_DSTL, dtype=np.float32)
        eww = np.zeros((T, P), dtype=np.float32)
        for b in range(NBLK):
            for half, K, cum, base in ((0, KA, cumKA, 0),
                                       (1, KB, cumKB, T_A)):
                idxs = lists[c][b][half]
                n = idxs.shape[0]
                g0 = base + cum[b]
                nslots = int(K[b]) * P
                # slot j (tile k=j//P, partition p=j%P) <- edge idxs[j]
                s_loc = np.zeros(nslots, dtype=np.int64)
                d_loc = np.zeros(nslots, dtype=np.int64)
                dl = np.full(nslots, PAD_DSTL, dtype=np.float32)
                w = np.zeros(nslots, dtype=np.float32)
                if n:
                    s_loc[:n] = src_s[idxs] - (HALF if half else 0)
                    d_loc[:n] = dst_s[idxs] - c * NPC
                    dl[:n] = (dst_s[idxs] - (c * NPC + b * P)).astype(
                        np.float32)
                    w[:n] = ew_s[idxs]
                sl2 = s_loc.reshape(int(K[b]), P)
                dl2 = d_loc.reshape(int(K[b]), P)
                dll2 = dl.reshape(int(K[b]), P)
                w2 = w.reshape(int(K[b]), P)
                src_rel[g0: g0 + int(K[b])] = sl2.astype(np.int16)
                dst_rel[g0: g0 + int(K[b])] = dl2.astype(np.int16)
                dstl[g0: g0 + int(K[b])] = dll2
                eww[g0: g0 + int(K[b])] = w2

        src_idx = np.zeros((n_chunks, P, IDX_COLS), dtype=np.int16)
        dst_idx = np.zeros((n_chunks, P, IDX_COLS), dtype=np.int16)
        for ci, ch in enumerate(chunks):
            g0, nt = ch["g0"], ch["nt"]
            # edge slot j = u*128 + p maps to tile g0+u, partition p
            flat_s = src_rel[g0: g0 + nt].reshape(nt * P)
            flat_d = dst_rel[g0: g0 + nt].reshape(nt * P)
            src_idx[ci] = _wrap_idx(flat_s)
            dst_idx[ci] = _wrap_idx(flat_d)

        per_core.append(dict(
            src_idx=src_idx, dst_idx=dst_idx,
            dstl=np.ascontiguousarray(dstl.T),   # [128, T]
            ew=np.ascontiguousarray(eww.T),      # [128, T]
        ))

    return plan, per_core


# build stages for HW bisection: 1=phase1+AG only, 2=+gathers,
# 3=+DVE score/S pipeline, 4=full (default)
BUILD_STAGE = 4
# repeat whole kernel body inside one NEFF (for timing by differencing)
REPS = 1


# ---------------------------------------------------------------- builder
def build(plan):
    n_chunks = plan["n_chunks"]
    chunks = plan["chunks"]
    T = plan["T"]

    nc = bacc.Bacc("TRN2", target_bir_lowering=False, debug=False,
                   num_devices=NCORES, num_swdge_queues=4)
    qctr = [0]

    # inputs
    x_t = nc.dram_tensor("x_t", [P, NPC], FP, kind="ExternalInput")
    w_in = nc.dram_tensor("w_in", [P, IN_F], FP, kind="ExternalInput")
    asrep = nc.dram_tensor("asrep", [P, IN_F], FP, kind="ExternalInput")
    adrep = nc.dram_tensor("adrep", [P, IN_F], FP, kind="ExternalInput")
    epwrep = nc.dram_tensor("epwrep", [P, H], FP, kind="ExternalInput")
    epbrep = nc.dram_tensor("epbrep", [P, H], FP, kind="ExternalInput")
    biasrep = nc.dram_tensor("biasrep", [P, OUT_F], FP, kind="ExternalInput")
    iotarep = nc.dram_tensor("iotarep", [P, P], FP, kind="ExternalInput")
    dstl_in = nc.dram_tensor("dstl_in", [P, T], FP, kind="ExternalInput")
    ew_in = nc.dram_tensor("ew_in", [P, T], FP, kind="ExternalInput")
    srcidx_in = nc.dram_tensor("srcidx_in", [n_chunks, P, IDX_COLS],
                               mybir.dt.int16, kind="ExternalInput")
    dstidx_in = nc.dram_tensor("dstidx_in", [n_chunks, P, IDX_COLS],
                               mybir.dt.int16, kind="ExternalInput")
    out = nc.dram_tensor("out", [NPC, OUT_F], FP, kind="ExternalOutput")

    with tile.TileContext(nc) as tc:
        for _rep in range(REPS):
            with tc.tile_pool(name="dram", bufs=1, space="DRAM") as dram, \
                 tc.tile_pool(name="statics", bufs=1) as statics:

                hs_in = dram.tile([NPC, ROW], FP)
                hs_full = dram.tile([NPAD, ROW], FP, addr_space="Shared")
                ad_pad = dram.tile([NPC, ADROW], FP)

                # ---------------- statics
                iota_sb = statics.tile([P, P], FP)
                nc.sync.dma_start(iota_sb[:], iotarep[:])
                epw_sb = statics.tile([P, H], FP)
                nc.sync.dma_start(epw_sb[:], epwrep[:])
                epb_sb = statics.tile([P, H], FP)
                nc.sync.dma_start(epb_sb[:], epbrep[:])
                bias_sb = statics.tile([P, OUT_F], FP)
                nc.sync.dma_start(bias_sb[:], biasrep[:])

                # ---------------- phase 1: node table
                with tc.tile_pool(name="p1", bufs=1) as p1, \
                     tc.tile_pool(name="p1psum", bufs=4, space="PSUM") as p1ps:
                    w_sb = p1.tile([P, IN_F], FP)
                    nc.sync.dma_start(w_sb[:], w_in[:])
                    as_sb = p1.tile([P, IN_F], FP)
                    nc.sync.dma_start(as_sb[:], asrep[:])
                    ad_sb = p1.tile([P, IN_F], FP)
                    nc.sync.dma_start(ad_sb[:], adrep[:])
                    xt_sb = p1.tile([P, NPC], FP)
                    nc.sync.dma_start(xt_sb[:], x_t[:])

                    # rhs_w = [W@a_dst | W | W@a_src]  [128, 144]
                    rhs_w = p1.tile([P, IN_F + 2 * H], FP)
                    nc.vector.tensor_copy(rhs_w[:, H: H + IN_F], w_sb[:])
                    tmp_d = p1.tile([P, IN_F], FP)
                    nc.vector.tensor_tensor(out=tmp_d[:], in0=w_sb[:],
                                            in1=ad_sb[:],
                                            op=mybir.AluOpType.mult)
                    nc.vector.tensor_reduce(
                        out=rhs_w[:, 0:H],
                        in_=tmp_d[:].rearrange("p (h d) -> p h d", d=HD),
                        axis=mybir.AxisListType.X, op=mybir.AluOpType.add)
                    tmp_s = p1.tile([P, IN_F], FP)
                    nc.vector.tensor_tensor(out=tmp_s[:], in0=w_sb[:],
                                            in1=as_sb[:],
                                            op=mybir.AluOpType.mult)
                    nc.vector.tensor_reduce(
                        out=rhs_w[:, H + IN_F: H + IN_F + H],
                        in_=tmp_s[:].rearrange("p (h d) -> p h d", d=HD),
                        axis=mybir.AxisListType.X, op=mybir.AluOpType.add)

                    hs_slice = p1.tile([P, NBLK * ROW], FP)
                    ad_slice = p1.tile([P, NBLK * ADROW], FP)
                    # zero the pad columns (never computed, but DMA'd/gathered)
                    nc.vector.memset(
                        hs_slice[:].rearrange("p (t r) -> p t r", r=ROW)
                        [:, :, AS_OFF + H: ROW], 0.0)
                    nc.vector.memset(ad_slice[:], 0.0)

                    for t in range(NBLK):
                        hpsum = p1ps.tile([P, IN_F + 2 * H], FP, space="PSUM")
                        nc.tensor.matmul(out=hpsum[:],
                                         lhsT=xt_sb[:, t * P: (t + 1) * P],
                                         rhs=rhs_w[:], start=True, stop=True)
                        # [ad | h | as] -> hs row gets [h|as], ad_slice gets ad
                        nc.vector.tensor_copy(
                            hs_slice[:, t * ROW: t * ROW + IN_F + H],
                            hpsum[:, H: 2 * H + IN_F])
                        nc.scalar.activation(
                            ad_slice[:, t * ADROW: t * ADROW + H],
                            hpsum[:, 0:H],
                            mybir.ActivationFunctionType.Copy)

                    nc.sync.dma_start(
                        hs_in[:].rearrange("(t p) r -> p t r", p=P),
                        hs_slice[:].rearrange("p (t r) -> p t r", r=ROW))
                    nc.sync.dma_start(
                        ad_pad[:].rearrange("(t p) h -> p t h", p=P),
                        ad_slice[:].rearrange("p (t h) -> p t h", h=ADROW))

                nc.gpsimd.collective_compute(
                    "AllGather", mybir.AluOpType.bypass,
                    replica_groups=[list(range(NCORES))],
                    ins=[hs_in[:]], outs=[hs_full[:]],
                )

                # ---------------- phase 2
                with tc.tile_pool(name="meta", bufs=1) as meta, \
                     tc.tile_pool(name="gp", bufs=4) as gp, \
                     tc.tile_pool(name="adp", bufs=4) as adp, \
                     tc.tile_pool(name="sp", bufs=4) as sp, \
                     tc.tile_pool(name="rp", bufs=4) as rp, \
                     tc.tile_pool(name="ep", bufs=2) as ep, \
                     tc.tile_pool(name="ip", bufs=4) as ip, \
                     tc.tile_pool(name="op", bufs=3) as opool, \
                     tc.tile_pool(name="bps", bufs=4, space="PSUM") as bps:

                    dstl_sb = meta.tile([P, T], FP)
                    nc.sync.dma_start(dstl_sb[:], dstl_in[:])
                    ew_sb = meta.tile([P, T], FP)
                    nc.sync.dma_start(ew_sb[:], ew_in[:])
                    sidx_all = meta.tile([P, n_chunks, IDX_COLS],
                                         mybir.dt.int16)
                    nc.sync.dma_start(
                        sidx_all[:],
                        srcidx_in[:].rearrange("c p i -> p c i"))
                    didx_all = meta.tile([P, n_chunks, IDX_COLS],
                                         mybir.dt.int16)
                    nc.sync.dma_start(
                        didx_all[:],
                        dstidx_in[:].rearrange("c p i -> p c i"))

                    chunk_tiles = {}

                    def emit_chunk(ci):
                        ch = chunks[ci]
                        g0, nt = ch["g0"], ch["nt"]
                        nidx = nt * P
                        n16 = nidx // 16
                        if BUILD_STAGE == 1:
                            return

                        sidx = sidx_all[:, ci, :]
                        didx = didx_all[:, ci, :]

                        gbuf = gp.tile([P, CHUNK_TILES, ROW], FP, tag="gbuf")
                        half_ap = (hs_full[0:HALF, :] if ch["stream"] == 0
                                   else hs_full[HALF:NPAD, :])
                        nc.gpsimd.dma_gather(
                            out_ap=gbuf[:, :nt, :], in_ap=half_ap,
                            idxs_ap=sidx[:, :n16],
                            num_idxs=nidx, num_idxs_reg=nidx, elem_size=ROW,
                            single_packet=False, queue_num=qctr[0] % 4)
                        qctr[0] += 1

                        adbuf = adp.tile([P, CHUNK_TILES, ADROW], FP,
                                         tag="adbuf")
                        nc.gpsimd.dma_gather(
                            out_ap=adbuf[:, :nt, :], in_ap=ad_pad[:],
                            idxs_ap=didx[:, :n16],
                            num_idxs=nidx, num_idxs_reg=nidx, elem_size=ADROW,
                            single_packet=False, queue_num=qctr[0] % 4)
                        qctr[0] += 1
                        if BUILD_STAGE == 2:
                            chunk_tiles[ci] = (gbuf, adbuf)
                            return

                        # one-hot S [P, nt, 128]
                        s_t = sp.tile([P, CHUNK_TILES * P], FP, tag="s_t")
                        s_v = s_t[:].rearrange("p (t n) -> p t n", n=P)
                        dstl_v = dstl_sb[:, g0: g0 + nt]
                        nc.vector.tensor_tensor(
                            out=s_v[:, :nt, :],
                            in0=dstl_v.unsqueeze(2).broadcast_to([P, nt, P]),
                            in1=iota_sb[:].unsqueeze(1).broadcast_to(
                                [P, nt, P]),
                            op=mybir.AluOpType.is_equal)

                        # scores e = leaky(as+ad) + ew*epw + epb ; p = exp(e)
                        e0 = ep.tile([P, CHUNK_TILES * H], FP, tag="e0")
                        e0v = e0[:].rearrange("p (t h) -> p t h", h=H)[:, :nt, :]
                        nc.vector.tensor_tensor(
                            out=e0v, in0=gbuf[:, :nt, AS_OFF: AS_OFF + H],
                            in1=adbuf[:, :nt, 0:H], op=mybir.AluOpType.add)
                        e1 = ep.tile([P, CHUNK_TILES * H], FP, tag="e1")
                        e1v = e1[:].rearrange("p (t h) -> p t h", h=H)[:, :nt, :]
                        nc.vector.tensor_scalar_mul(out=e1v, in0=e0v,
                                                    scalar1=ALPHA)
                        e2 = ep.tile([P, CHUNK_TILES * H], FP, tag="e2")
                        e2v = e2[:].rearrange("p (t h) -> p t h", h=H)[:, :nt, :]
                        nc.vector.tensor_tensor(out=e2v, in0=e0v, in1=e1v,
                                                op=mybir.AluOpType.max)
                        # ew*epw + epb
                        e3 = ep.tile([P, CHUNK_TILES * H], FP, tag="e3")
                        e3v = e3[:].rearrange("p (t h) -> p t h", h=H)[:, :nt, :]
                        nc.vector.tensor_tensor(
                            out=e3v,
                            in0=ew_sb[:, g0: g0 + nt].unsqueeze(2).broadcast_to(
                                [P, nt, H]),
                            in1=epw_sb[:].unsqueeze(1).broadcast_to([P, nt, H]),
                            op=mybir.AluOpType.mult)
                        e4 = ep.tile([P, CHUNK_TILES * H], FP, tag="e4")
                        e4v = e4[:].rearrange("p (t h) -> p t h", h=H)[:, :nt, :]
                        nc.vector.tensor_tensor(out=e4v, in0=e3v,
                                                in1=epb_sb[:].unsqueeze(1)
                                                .broadcast_to([P, nt, H]),
                                                op=mybir.AluOpType.add)
                        e5 = ep.tile([P, CHUNK_TILES * H], FP, tag="e5")
                        e5v = e5[:].rearrange("p (t h) -> p t h", h=H)[:, :nt, :]
                        nc.vector.tensor_tensor(out=e5v, in0=e2v, in1=e4v,
                                                op=mybir.AluOpType.add)

                        # rhs tile: [msgs(128) | p(8)] per tile
                        rhs = rp.tile([P, CHUNK_TILES * (OUT_F + H)], FP,
                                      tag="rhs")
                        rhs_v = rhs[:].rearrange("p (t f) -> p t f",
                                                 f=OUT_F + H)
                        nc.scalar.activation(
                            rhs_v[:, :nt, OUT_F: OUT_F + H], e5v,
                            mybir.ActivationFunctionType.Exp)
                        # msgs = h * p (broadcast over head dim)
                        nc.vector.tensor_tensor(
                            out=rhs_v[:, :nt, 0:OUT_F].rearrange(
                                "p t (h d) -> p t h d", d=HD),
                            in0=gbuf[:, :nt, 0:IN_F].rearrange(
                                "p t (h d) -> p t h d", d=HD),
                            in1=rhs_v[:, :nt, OUT_F: OUT_F + H].unsqueeze(3)
                                .broadcast_to([P, nt, H, HD]),
                            op=mybir.AluOpType.mult)
                        chunk_tiles[ci] = (s_t, rhs)

                    if BUILD_STAGE < 4:
                        # bisection modes: run phase-2 pieces, dump something
                        for ci in range(n_chunks):
                            emit_chunk(ci)
                        dump = opool.tile([P, OUT_F], FP, tag="dump")
                        if BUILD_STAGE == 1:
                            nc.vector.memset(dump[:], 0.0)
                        elif BUILD_STAGE == 2:
                            g0buf = chunk_tiles[0][0]
                            nc.vector.tensor_copy(dump[:], g0buf[:, 0, 0:OUT_F])
                        else:
                            r0 = chunk_tiles[0][1]
                            nc.vector.tensor_copy(dump[:], r0[:, 0:OUT_F])
                        for b in range(NBLK):
                            nc.sync.dma_start(out[b * P: (b + 1) * P, :],
                                              dump[:])

                    for b in range(NBLK if BUILD_STAGE >= 4 else 0):
                        tl = plan["block_tiles"][b]
                        for (ci, slot) in tl:
                            if ci not in chunk_tiles:
                                emit_chunk(ci)
                        psum_b = bps.tile([P, OUT_F + H], FP, space="PSUM",
                                          tag="psum_b")
                        for i, (ci, slot) in enumerate(tl):
                            s_t, rhs = chunk_tiles[ci]
                            nc.tensor.matmul(
                                out=psum_b[:],
                                lhsT=s_t[:, slot * P: (slot + 1) * P],
                                rhs=rhs[:, slot * (OUT_F + H):
                                        (slot + 1) * (OUT_F + H)],
                                start=(i == 0), stop=(i == len(tl) - 1))
                        # normalize + bias
                        s_eps = opool.tile([P, H], FP, tag="s_eps")
                        nc.vector.tensor_scalar_add(
                            out=s_eps[:], in0=psum_b[:, OUT_F: OUT_F + H],
                            scalar1=EPS)
                        rcp = opool.tile([P, H], FP, tag="rcp")
                        nc.vector.reciprocal(rcp[:], s_eps[:])
                        ob1 = opool.tile([P, OUT_F], FP, tag="ob1")
                        nc.vector.tensor_tensor(
                            out=ob1[:].rearrange("p (h d) -> p h d", d=HD),
                            in0=psum_b[:, 0:OUT_F].rearrange(
                                "p (h d) -> p h d", d=HD),
                            in1=rcp[:].unsqueeze(2).broadcast_to([P, H, HD]),
                            op=mybir.AluOpType.mult)
                        ob2 = opool.tile([P, OUT_F], FP, tag="ob2")
                        nc.vector.tensor_tensor(out=ob2[:], in0=ob1[:],
                                                in1=bias_sb[:],
                                                op=mybir.AluOpType.add)
                        nc.sync.dma_start(out[b * P: (b + 1) * P, :], ob2[:])

    nc.compile()
    # SWDGE constraint: a DMA semaphore may only be updated from one queue.
    # Tile assigns DMASW lanes post-scheduling, so align queue_num to lane.
    for f in nc.m.functions:
        for bb in f.blocks:
            for ins in bb.instructions:
                if type(ins).__name__ == "InstDMAGatherAnt":
                    si = ins.sync_info
                    lane = None
                    for u in si.on_update:
                        nm = u.ant_name or ""
                        if nm.startswith("DMASW"):
                            lane = int(nm[5:].split("_")[0])
                            break
                    assert lane is not None, "gather without DMASW sem"
                    ins.queue_num = lane % 4
    return nc


# ---------------------------------------------------------------- host API
def make_in_maps(x, W, a_src, a_dst, ep_w, ep_b, bias, per_core):
    x = np.asarray(x, dtype=np.float32)
    W = np.asarray(W, dtype=np.float32)
    a_src = np.asarray(a_src, dtype=np.float32)
    a_dst = np.asarray(a_dst, dtype=np.float32)
    ep_w = np.asarray(ep_w, dtype=np.float32)
    ep_b = np.asarray(ep_b, dtype=np.float32)
    bias = np.asarray(bias, dtype=np.float32)

    x_pad = np.zeros((NPAD, IN_F), dtype=np.float32)
    x_pad[:N] = x
    # W [H, IN, HD] -> [IN, H*HD]
    w_flat = np.ascontiguousarray(W.transpose(1, 0, 2).reshape(IN_F, H * HD))
    as_flat = a_src.reshape(H * HD).astype(np.float32)
    ad_flat = a_dst.reshape(H * HD).astype(np.float32)

    rep = lambda v: np.ascontiguousarray(
        np.broadcast_to(v[None, :], (P, v.shape[0])))
    iota = np.broadcast_to(np.arange(P, dtype=np.float32)[None, :], (P, P))

    maps = []
    for c in range(NCORES):
        pc = per_core[c]
        x_t = np.ascontiguousarray(x_pad[c * NPC: (c + 1) * NPC, :].T)
        maps.append({
            "x_t": x_t,
            "w_in": w_flat,
            "asrep": rep(as_flat),
            "adrep": rep(ad_flat),
            "epwrep": rep(ep_w),
            "epbrep": rep(ep_b),
            "biasrep": rep(bias),
            "iotarep": np.ascontiguousarray(iota),
            "dstl_in": pc["dstl"],
            "ew_in": pc["ew"],
            "srcidx_in": pc["src_idx"],
            "dstidx_in": pc["dst_idx"],
        })
    return maps


_CACHE = {}


def kernel(x, edge_index, edge_weight, W, a_src, a_dst, ep_w, ep_b, bias):
    import hashlib
    key = hashlib.sha1(
        np.ascontiguousarray(np.asarray(edge_index, dtype=np.int64))
    ).hexdigest()
    if key not in _CACHE:
        plan, per_core = plan_and_inputs(edge_index, edge_weight)
        nc = build(plan)
        _CACHE[key] = (plan, per_core, nc)
    plan, per_core, nc = _CACHE[key]

    in_maps = make_in_maps(x, W, a_src, a_dst, ep_w, ep_b, bias, per_core)
    res = run_bass_kernel_spmd(nc, in_maps, core_ids=list(range(NCORES)),
                               trace=False)
    out_full = np.empty((NPAD, OUT_F), dtype=np.float32)
    for c in range(NCORES):
        out_full[c * NPC: (c + 1) * NPC] = res.results[c]["out"]
    return out_full[:N]



# revision 3
# speedup vs baseline: 1.2113x; 1.2113x over previous
"""EnhancedCorrelationGNN Trainium2 kernel (8 NeuronCores, SPMD).

Strategy: destination-sorted edge processing with node-range output sharding,
fully collective-free.
 - Host (free): counting-sort edges by dst, partition nodes into 8 ranges of
   6272 (49 blocks x 128 nodes per core). Per core the node table is ROTATED
   so its own slice comes first; per block, edges are split by rotated src
   half (dma_gather int16 index limit) and padded to 128-edge tiles with
   cross-core-uniform tile counts (one SPMD program).
 - Phase 1 (device): EVERY core computes the FULL node table from the
   replicated x input: h = x @ W plus both attention projections in ONE bf16
   matmul per 128-node tile (rhs = [W@a_dst | W | W@a_src] prepped on host),
   writes bf16 [h|as] rows (512B) to local DRAM. attn_d for the core's own
   49 blocks stays in SBUF. No AllGather.
 - Phase 2 (device): per 32-tile chunk, one dma_gather of bf16 [h|as] rows
   by src (512B/edge); attn_d is expanded per-edge by a TensorE matmul with
   a host-shipped fp8 one-hot (node x edge) instead of a second gather.
   Scores: DVE adds + ACT Lrelu/Exp; messages bf16; one-hot segment matrix
   via is_equal(dstl, iota) in bf16; per-tile bf16 TensorE matmul
   scatter-accumulates [msgs | p] into the block PSUM; per block normalize
   by 1/(sum p + 1e-10), add bias, DMA out.
"""
import sys

if "/opt/trn_rl_repo" not in sys.path:
    sys.path.insert(0, "/opt/trn_rl_repo")

import numpy as np
import ml_dtypes

import concourse.bass as bass
import concourse.bacc as bacc
import concourse.mybir as mybir
import concourse.tile as tile
from concourse.bass_utils import run_bass_kernel_spmd

# ---------------------------------------------------------------- constants
N = 50000
E = 800000
IN_F = 128
H = 8
HD = 16
OUT_F = H * HD          # 128
ALPHA = 0.2
EPS = 1e-10

NCORES = 8
P = 128
NPC = 6272              # nodes per core = 49 * 128; 8*6272 = 50176 >= N
NPAD = NCORES * NPC     # 50176
NBLK = NPC // P         # 49
HALF = NPAD // 2        # 25088 rotated-table rows per gather stream

ROW = 256               # table row elems (bf16): h(128) | as(8) | pad -> 512B
AS_OFF = 128            # attn_s offset within row
CHUNK_TILES = 32        # tiles per gather/DVE chunk
IDX_COLS = CHUNK_TILES * P // 16   # wrapped int16 idx columns per chunk
PAD_DSTL = 300.0        # one-hot miss sentinel (exact in bf16)
XBLK = 28               # phase-1 blocks per xT chunk; 392 = 14 * 28
NXCH = (NPAD // P) // XBLK         # 14 phase-1 chunks (7 per half)

FP = mybir.dt.float32
BF = mybir.dt.bfloat16
F8 = mybir.dt.float8e4
NP_BF = ml_dtypes.bfloat16
NP_F8 = ml_dtypes.float8_e4m3

USE_FP8_S2 = True       # one-hot S2 dtype (fp8 halves its DMA vs bf16)


# ---------------------------------------------------------------- planning
def _cdiv(a, b):
    return -(-a // b)


def _wrap_idx(idx_flat: np.ndarray) -> np.ndarray:
    """[n] -> [128, IDX_COLS] int16: idx j at [j%16, j//16], replicated x8."""
    n = idx_flat.shape[0]
    assert n % 16 == 0
    w = idx_flat.reshape(n // 16, 16).T.astype(np.int16)      # [16, n/16]
    w = np.tile(w, (8, 1))                                    # [128, n/16]
    out = np.zeros((P, IDX_COLS), dtype=np.int16)
    out[:, : w.shape[1]] = w
    return out


def plan_and_inputs(edge_index, edge_weight):
    """Host-side edge partitioning. Returns (plan, per_core_arrays).

    plan (core-independent, defines the SPMD program):
      KA, KB: [NBLK] tiles per (block, half)
      chunks: list of dicts(stream, g0, nt) over stream-major tile ids
      block_tiles: per block, list of (chunk_id, slot) in matmul order
      tile_block: [T] block id of each global tile
      T, n_chunks
    per_core_arrays[c]:
      src_idx [n_chunks,128,IDX_COLS] i16 (stream-relative, rotated table)
      dstl    [128, T] bf16 (block-relative dst, PAD_DSTL for pad slots)
      ew      [128, T] bf16
      s2      [128, T*128] fp8/bf16 one-hot: s2[n, t*128+e] = (dstl[e,t]==n)
    """
    src = np.asarray(edge_index[0], dtype=np.int64)
    dst = np.asarray(edge_index[1], dtype=np.int64)
    ew = np.asarray(edge_weight, dtype=np.float32)

    order = np.argsort(dst, kind="stable")
    src_s, dst_s, ew_s = src[order], dst[order], ew[order]

    # block boundaries over sorted dst
    blk_starts = np.searchsorted(dst_s, np.arange(0, NPAD + 1, P))
    # per (core, block, half) edge index lists (into the sorted arrays)
    cnt = np.zeros((NCORES, NBLK, 2), dtype=np.int64)
    lists = [[[None, None] for _ in range(NBLK)] for _ in range(NCORES)]
    rot_all = []
    for c in range(NCORES):
        rot = (src_s - c * NPC) % NPAD     # rotated src row per core
        rot_all.append(rot)
        for b in range(NBLK):
            g = c * NBLK + b
            lo, hi = blk_starts[g], blk_starts[g + 1]
            r = rot[lo:hi]
            mA = r < HALF
            idxs = np.arange(lo, hi)
            lists[c][b][0] = idxs[mA]
            lists[c][b][1] = idxs[~mA]
            cnt[c, b, 0] = mA.sum()
            cnt[c, b, 1] = (~mA).sum()

    KA = np.maximum(_cdiv(cnt[:, :, 0].max(axis=0), P), 1).astype(np.int64)
    KB = _cdiv(cnt[:, :, 1].max(axis=0), P).astype(np.int64)

    T_A = int(KA.sum())
    T_B = int(KB.sum())
    T = T_A + T_B
    cumKA = np.concatenate([[0], np.cumsum(KA)])
    cumKB = np.concatenate([[0], np.cumsum(KB)])

    # chunks: stream-major [0,T_A) then [T_A,T)
    chunks = []
    g = 0
    while g < T_A:
        nt = min(CHUNK_TILES, T_A - g)
        chunks.append(dict(stream=0, g0=g, nt=nt))
        g += nt
    while g < T:
        nt = min(CHUNK_TILES, T - g)
        chunks.append(dict(stream=1, g0=g, nt=nt))
        g += nt
    n_chunks = len(chunks)

    chunk_of = np.empty(T, dtype=np.int64)
    slot_of = np.empty(T, dtype=np.int64)
    for ci, ch in enumerate(chunks):
        chunk_of[ch["g0"]: ch["g0"] + ch["nt"]] = ci
        slot_of[ch["g0"]: ch["g0"] + ch["nt"]] = np.arange(ch["nt"])

    tile_block = np.empty(T, dtype=np.int64)
    block_tiles = []
    for b in range(NBLK):
        tl = []
        for k in range(KA[b]):
            gidx = cumKA[b] + k
            tile_block[gidx] = b
            tl.append((int(chunk_of[gidx]), int(slot_of[gidx])))
        for k in range(KB[b]):
            gidx = T_A + cumKB[b] + k
            tile_block[gidx] = b
            tl.append((int(chunk_of[gidx]), int(slot_of[gidx])))
        block_tiles.append(tl)

    plan = dict(KA=KA, KB=KB, T=T, T_A=T_A, chunks=chunks,
                block_tiles=block_tiles, tile_block=tile_block,
                n_chunks=n_chunks)

    # ---------------- per-core slot arrays
    s2_dt = NP_F8 if USE_FP8_S2 else NP_BF
    per_core = []
    for c in range(NCORES):
        src_rel = np.zeros((T, P), dtype=np.int16)
        dstl = np.full((T, P), PAD_DSTL, dtype=np.float32)
        eww = np.zeros((T, P), dtype=np.float32)
        rot = rot_all[c]
        for b in range(NBLK):
            for half, K, cum, base in ((0, KA, cumKA, 0),
                                       (1, KB, cumKB, T_A)):
                idxs = lists[c][b][half]
                n = idxs.shape[0]
                g0 = base + cum[b]
                nslots = int(K[b]) * P
                s_loc = np.zeros(nslots, dtype=np.int64)
                dl = np.full(nslots, PAD_DSTL, dtype=np.float32)
                w = np.zeros(nslots, dtype=np.float32)
                if n:
                    s_loc[:n] = rot[idxs] - (HALF if half else 0)
                    dl[:n] = (dst_s[idxs] - (c * NPC + b * P)).astype(
                        np.float32)
                    w[:n] = ew_s[idxs]
                src_rel[g0: g0 + int(K[b])] = s_loc.reshape(
                    int(K[b]), P).astype(np.int16)
                dstl[g0: g0 + int(K[b])] = dl.reshape(int(K[b]), P)
                eww[g0: g0 + int(K[b])] = w.reshape(int(K[b]), P)

        src_idx = np.zeros((n_chunks, P, IDX_COLS), dtype=np.int16)
        for ci, ch in enumerate(chunks):
            g0, nt = ch["g0"], ch["nt"]
            src_idx[ci] = _wrap_idx(src_rel[g0: g0 + nt].reshape(nt * P))

        # one-hot S2 [node, T*128 edge slots]; PAD_DSTL rows match nothing
        s2 = (np.arange(P, dtype=np.float32)[:, None]
              == dstl.reshape(T * P)[None, :]).astype(s2_dt)

        per_core.append(dict(
            src_idx=src_idx,
            dstl=np.ascontiguousarray(dstl.T).astype(NP_BF),   # [128, T]
            ew=np.ascontiguousarray(eww.T).astype(NP_BF),      # [128, T]
            s2=np.ascontiguousarray(s2),                       # [128, T*128]
        ))

    return plan, per_core


# build stages for HW bisection: 1=phase1+table only, 2=+gathers+s2,
# 3=+score/rhs pipeline, 4=full (default)
BUILD_STAGE = 4
# repeat whole kernel body inside one NEFF (for timing by differencing)
REPS = 1


# ---------------------------------------------------------------- builder
def build(plan):
    n_chunks = plan["n_chunks"]
    chunks = plan["chunks"]
    T = plan["T"]
    tile_block = plan["tile_block"]

    nc = bacc.Bacc("TRN2", target_bir_lowering=False, debug=False,
                   num_devices=NCORES, num_swdge_queues=4)
    qctr = [0]
    S2DT = F8 if USE_FP8_S2 else BF

    # inputs
    xt_in = nc.dram_tensor("xt_in", [P, NPAD], BF, kind="ExternalInput")
    rhsw_in = nc.dram_tensor("rhsw_in", [P, IN_F + 2 * H], BF,
                             kind="ExternalInput")
    epw_in = nc.dram_tensor("epw_in", [P, H], BF, kind="ExternalInput")
    epb_in = nc.dram_tensor("epb_in", [P, H], BF, kind="ExternalInput")
    bias_in = nc.dram_tensor("bias_in", [P, OUT_F], FP, kind="ExternalInput")
    iota_in = nc.dram_tensor("iota_in", [P, P], BF, kind="ExternalInput")
    dstl_in = nc.dram_tensor("dstl_in", [P, T], BF, kind="ExternalInput")
    ew_in = nc.dram_tensor("ew_in", [P, T], BF, kind="ExternalInput")
    sidx_in = nc.dram_tensor("sidx_in", [n_chunks, P, IDX_COLS],
                             mybir.dt.int16, kind="ExternalInput")
    s2_in = nc.dram_tensor("s2_in", [P, T * P], S2DT, kind="ExternalInput")
    out = nc.dram_tensor("out", [NPC, OUT_F], FP, kind="ExternalOutput")

    with tile.TileContext(nc) as tc:
        for _rep in range(REPS):
            with tc.tile_pool(name="dram", bufs=1, space="DRAM") as dram, \
                 tc.tile_pool(name="statics", bufs=1) as statics:

                hs_A = dram.tile([HALF, ROW], BF)
                hs_B = dram.tile([HALF, ROW], BF)

                # ---------------- statics
                iota_sb = statics.tile([P, P], BF)
                nc.sync.dma_start(iota_sb[:], iota_in[:])
                epw_sb = statics.tile([P, H], BF)
                nc.sync.dma_start(epw_sb[:], epw_in[:])
                epb_sb = statics.tile([P, H], BF)
                nc.sync.dma_start(epb_sb[:], epb_in[:])
                bias_sb = statics.tile([P, OUT_F], FP)
                nc.sync.dma_start(bias_sb[:], bias_in[:])
                rhsw_sb = statics.tile([P, IN_F + 2 * H], BF)
                nc.sync.dma_start(rhsw_sb[:], rhsw_in[:])
                # attn_d for this core's own 49 blocks (rotated blocks 0..48)
                ad_blk = statics.tile([P, NBLK * H], BF)

                # ---------------- phase 1: full node table, local
                with tc.tile_pool(name="p1x", bufs=2) as p1x, \
                     tc.tile_pool(name="p1s", bufs=3) as p1s, \
                     tc.tile_pool(name="p1ps", bufs=4, space="PSUM") as p1ps:
                    for ch in range(NXCH):
                        xc = p1x.tile([P, XBLK * P], BF, tag="xc")
                        nc.sync.dma_start(
                            xc[:], xt_in[:, ch * XBLK * P:
                                         (ch + 1) * XBLK * P])
                        stage = p1s.tile([P, XBLK * ROW], BF, tag="stage")
                        for k in range(XBLK):
                            g = ch * XBLK + k
                            hpsum = p1ps.tile([P, IN_F + 2 * H], FP,
                                              space="PSUM")
                            nc.tensor.matmul(out=hpsum[:],
                                             lhsT=xc[:, k * P: (k + 1) * P],
                                             rhs=rhsw_sb[:],
                                             start=True, stop=True)
                            # [ad | h | as] -> row [h|as], own-slice ad kept
                            nc.scalar.activation(
                                stage[:, k * ROW: k * ROW + IN_F + H],
                                hpsum[:, H: 2 * H + IN_F],
                                mybir.ActivationFunctionType.Copy)
                            if g < NBLK:
                                nc.scalar.activation(
                                    ad_blk[:, g * H: (g + 1) * H],
                                    hpsum[:, 0:H],
                                    mybir.ActivationFunctionType.Copy)
                        half_t = hs_A if ch < NXCH // 2 else hs_B
                        r0 = (ch % (NXCH // 2)) * XBLK * P
                        nc.sync.dma_start(
                            half_t[r0: r0 + XBLK * P, :].rearrange(
                                "(t p) f -> p t f", p=P),
                            stage[:].rearrange("p (t f) -> p t f", f=ROW))

                # ---------------- phase 2
                with tc.tile_pool(name="meta", bufs=1) as meta, \
                     tc.tile_pool(name="gp", bufs=3) as gp, \
                     tc.tile_pool(name="s2p", bufs=3) as s2p, \
                     tc.tile_pool(name="sp", bufs=3) as sp, \
                     tc.tile_pool(name="rp", bufs=3) as rp, \
                     tc.tile_pool(name="ep", bufs=3) as ep, \
                     tc.tile_pool(name="op", bufs=3) as opool, \
                     tc.tile_pool(name="adps", bufs=2, space="PSUM") as adps, \
                     tc.tile_pool(name="bps", bufs=4, space="PSUM") as bps:

                    dstl_sb = meta.tile([P, T], BF)
                    nc.sync.dma_start(dstl_sb[:], dstl_in[:])
                    ew_sb = meta.tile([P, T], BF)
                    nc.sync.dma_start(ew_sb[:], ew_in[:])
                    sidx_all = meta.tile([P, n_chunks, IDX_COLS],
                                         mybir.dt.int16)
                    nc.sync.dma_start(
                        sidx_all[:],
                        sidx_in[:].rearrange("c p i -> p c i"))

                    # e4_all = ew*epw + epb for every slot, upfront
                    e4_all = meta.tile([P, T * H], BF)
                    e4v_all = e4_all[:].rearrange("p (t h) -> p t h", h=H)
                    nc.vector.tensor_tensor(
                        out=e4v_all,
                        in0=ew_sb[:].unsqueeze(2).broadcast_to([P, T, H]),
                        in1=epw_sb[:].unsqueeze(1).broadcast_to([P, T, H]),
                        op=mybir.AluOpType.mult)
                    nc.vector.tensor_tensor(
                        out=e4v_all, in0=e4v_all,
                        in1=epb_sb[:].unsqueeze(1).broadcast_to([P, T, H]),
                        op=mybir.AluOpType.add)

                    chunk_tiles = {}

                    def emit_chunk(ci):
                        ch = chunks[ci]
                        g0, nt = ch["g0"], ch["nt"]
                        nidx = nt * P
                        n16 = nidx // 16
                        if BUILD_STAGE == 1:
                            return

                        gbuf = gp.tile([P, CHUNK_TILES, ROW], BF, tag="gbuf")
                        half_ap = (hs_A[:] if ch["stream"] == 0 else hs_B[:])
                        nc.gpsimd.dma_gather(
                            out_ap=gbuf[:, :nt, :], in_ap=half_ap,
                            idxs_ap=sidx_all[:, ci, :n16],
                            num_idxs=nidx, num_idxs_reg=nidx, elem_size=ROW,
                            single_packet=False, queue_num=qctr[0] % 4)
                        qctr[0] += 1

                        s2c = s2p.tile([P, CHUNK_TILES * P], S2DT, tag="s2c")
                        nc.sync.dma_start(
                            s2c[:, : nt * P],
                            s2_in[:, g0 * P: (g0 + nt) * P])
                        if BUILD_STAGE == 2:
                            chunk_tiles[ci] = (gbuf, s2c)
                            return

                        # ad_edge[e, h] per tile via one-hot matmul
                        adp = adps.tile([P, CHUNK_TILES * H], FP,
                                        space="PSUM", tag="adp")
                        for t in range(nt):
                            b = int(tile_block[g0 + t])
                            nc.tensor.matmul(
                                out=adp[:, t * H: (t + 1) * H],
                                lhsT=s2c[:, t * P: (t + 1) * P],
                                rhs=ad_blk[:, b * H: (b + 1) * H],
                                start=True, stop=True)

                        # one-hot S [P, nt, 128]
                        s_t = sp.tile([P, CHUNK_TILES * P], BF, tag="s_t")
                        s_v = s_t[:].rearrange("p (t n) -> p t n", n=P)
                        dstl_v = dstl_sb[:, g0: g0 + nt]
                        nc.vector.tensor_tensor(
                            out=s_v[:, :nt, :],
                            in0=dstl_v.unsqueeze(2).broadcast_to([P, nt, P]),
                            in1=iota_sb[:].unsqueeze(1).broadcast_to(
                                [P, nt, P]),
                            op=mybir.AluOpType.is_equal)

                        # scores: e0 = as + ad ; e2 = lrelu(e0) ;
                        # e5 = e2 + e4 ; p = exp(e5)
                        e0 = ep.tile([P, CHUNK_TILES * H], BF, tag="e0")
                        e0v = e0[:].rearrange("p (t h) -> p t h",
                                              h=H)[:, :nt, :]
                        nc.vector.tensor_tensor(
                            out=e0v, in0=gbuf[:, :nt, AS_OFF: AS_OFF + H],
                            in1=adp[:, : nt * H].rearrange(
                                "p (t h) -> p t h", h=H),
                            op=mybir.AluOpType.add)
                        e1 = ep.tile([P, CHUNK_TILES * H], BF, tag="e1")
                        e1v = e1[:].rearrange("p (t h) -> p t h",
                                              h=H)[:, :nt, :]
                        nc.vector.tensor_scalar_mul(out=e1v, in0=e0v,
                                                    scalar1=ALPHA)
                        e2 = ep.tile([P, CHUNK_TILES * H], BF, tag="e2")
                        e2v = e2[:].rearrange("p (t h) -> p t h",
                                              h=H)[:, :nt, :]
                        nc.vector.tensor_tensor(out=e2v, in0=e0v, in1=e1v,
                                                op=mybir.AluOpType.max)
                        e5 = ep.tile([P, CHUNK_TILES * H], BF, tag="e5")
                        e5v = e5[:].rearrange("p (t h) -> p t h",
                                              h=H)[:, :nt, :]
                        nc.vector.tensor_tensor(
                            out=e5v, in0=e2v,
                            in1=e4v_all[:, g0: g0 + nt, :],
                            op=mybir.AluOpType.add)

                        # rhs tile: [msgs(128) | p(8)] per tile
                        rhs = rp.tile([P, CHUNK_TILES * (OUT_F + H)], BF,
                                      tag="rhs")
                        rhs_v = rhs[:].rearrange("p (t f) -> p t f",
                                                 f=OUT_F + H)
                        nc.scalar.activation(
                            rhs_v[:, :nt, OUT_F: OUT_F + H], e5v,
                            mybir.ActivationFunctionType.Exp)
                        # msgs = h * p (broadcast over head dim)
                        nc.vector.tensor_tensor(
                            out=rhs_v[:, :nt, 0:OUT_F].rearrange(
                                "p t (h d) -> p t h d", d=HD),
                            in0=gbuf[:, :nt, 0:IN_F].rearrange(
                                "p t (h d) -> p t h d", d=HD),
                            in1=rhs_v[:, :nt, OUT_F: OUT_F + H].unsqueeze(3)
                                .broadcast_to([P, nt, H, HD]),
                            op=mybir.AluOpType.mult)
                        chunk_tiles[ci] = (s_t, rhs)

                    if BUILD_STAGE < 4:
                        for ci in range(n_chunks):
                            emit_chunk(ci)
                        dump = opool.tile([P, OUT_F], FP, tag="dump")
                        if BUILD_STAGE == 1:
                            nc.vector.memset(dump[:], 0.0)
                        elif BUILD_STAGE == 2:
                            g0buf = chunk_tiles[0][0]
                            nc.vector.tensor_copy(dump[:],
                                                  g0buf[:, 0, 0:OUT_F])
                        else:
                            r0 = chunk_tiles[0][1]
                            nc.vector.tensor_copy(dump[:], r0[:, 0:OUT_F])
                        for b in range(NBLK):
                            nc.sync.dma_start(out[b * P: (b + 1) * P, :],
                                              dump[:])

                    for b in range(NBLK if BUILD_STAGE >= 4 else 0):
                        tl = plan["block_tiles"][b]
                        for (ci, slot) in tl:
                            if ci not in chunk_tiles:
                                emit_chunk(ci)
                        psum_b = bps.tile([P, OUT_F + H], FP, space="PSUM",
                                          tag="psum_b")
                        for i, (ci, slot) in enumerate(tl):
                            s_t, rhs = chunk_tiles[ci]
                            nc.tensor.matmul(
                                out=psum_b[:],
                                lhsT=s_t[:, slot * P: (slot + 1) * P],
                                rhs=rhs[:, slot * (OUT_F + H):
                                        (slot + 1) * (OUT_F + H)],
                                start=(i == 0), stop=(i == len(tl) - 1))
                        # normalize + bias
                        s_eps = opool.tile([P, H], FP, tag="s_eps")
                        nc.vector.tensor_scalar_add(
                            out=s_eps[:], in0=psum_b[:, OUT_F: OUT_F + H],
                            scalar1=EPS)
                        rcp = opool.tile([P, H], FP, tag="rcp")
                        nc.vector.reciprocal(rcp[:], s_eps[:])
                        ob1 = opool.tile([P, OUT_F], FP, tag="ob1")
                        nc.vector.tensor_tensor(
                            out=ob1[:].rearrange("p (h d) -> p h d", d=HD),
                            in0=psum_b[:, 0:OUT_F].rearrange(
                                "p (h d) -> p h d", d=HD),
                            in1=rcp[:].unsqueeze(2).broadcast_to([P, H, HD]),
                            op=mybir.AluOpType.mult)
                        ob2 = opool.tile([P, OUT_F], FP, tag="ob2")
                        nc.vector.tensor_tensor(out=ob2[:], in0=ob1[:],
                                                in1=bias_sb[:],
                                                op=mybir.AluOpType.add)
                        nc.sync.dma_start(out[b * P: (b + 1) * P, :], ob2[:])

    nc.compile()
    # SWDGE constraint: a DMA semaphore may only be updated from one queue.
    # Tile assigns DMASW lanes post-scheduling, so align queue_num to lane.
    for f in nc.m.functions:
        for bb in f.blocks:
            for ins in bb.instructions:
                if type(ins).__name__ == "InstDMAGatherAnt":
                    si = ins.sync_info
                    lane = None
                    for u in si.on_update:
                        nm = u.ant_name or ""
                        if nm.startswith("DMASW"):
                            lane = int(nm[5:].split("_")[0])
                            break
                    assert lane is not None, "gather without DMASW sem"
                    ins.queue_num = lane % 4
    return nc


# ---------------------------------------------------------------- host API
def make_in_maps(x, W, a_src, a_dst, ep_w, ep_b, bias, per_core):
    x = np.asarray(x, dtype=np.float32)
    W = np.asarray(W, dtype=np.float32)
    a_src = np.asarray(a_src, dtype=np.float32)
    a_dst = np.asarray(a_dst, dtype=np.float32)
    ep_w = np.asarray(ep_w, dtype=np.float32)
    ep_b = np.asarray(ep_b, dtype=np.float32)
    bias = np.asarray(bias, dtype=np.float32)

    x_pad = np.zeros((NPAD, IN_F), dtype=np.float32)
    x_pad[:N] = x
    # rhs_w = [W@a_dst | W | W@a_src]  [IN, 144]
    w_flat = W.transpose(1, 0, 2).reshape(IN_F, H * HD)       # [IN, H*HD]
    wad = np.einsum('hid,hd->ih', W, a_dst)                   # [IN, H]
    was = np.einsum('hid,hd->ih', W, a_src)                   # [IN, H]
    rhs_w = np.concatenate([wad, w_flat, was], axis=1).astype(NP_BF)

    rep = lambda v, dt: np.ascontiguousarray(
        np.broadcast_to(v[None, :], (P, v.shape[0]))).astype(dt)
    iota = np.broadcast_to(np.arange(P, dtype=np.float32)[None, :], (P, P))

    maps = []
    for c in range(NCORES):
        pc = per_core[c]
        x_rot = np.roll(x_pad, -c * NPC, axis=0)
        xt = np.ascontiguousarray(x_rot.T).astype(NP_BF)
        maps.append({
            "xt_in": xt,
            "rhsw_in": np.ascontiguousarray(rhs_w),
            "epw_in": rep(ep_w, NP_BF),
            "epb_in": rep(ep_b, NP_BF),
            "bias_in": rep(bias, np.float32),
            "iota_in": np.ascontiguousarray(iota).astype(NP_BF),
            "dstl_in": pc["dstl"],
            "ew_in": pc["ew"],
            "sidx_in": pc["src_idx"],
            "s2_in": pc["s2"],
        })
    return maps


_CACHE = {}


def kernel(x, edge_index, edge_weight, W, a_src, a_dst, ep_w, ep_b, bias):
    import hashlib
    key = hashlib.sha1(
        np.ascontiguousarray(np.asarray(edge_index, dtype=np.int64))
    ).hexdigest()
    if key not in _CACHE:
        plan, per_core = plan_and_inputs(edge_index, edge_weight)
        nc = build(plan)
        _CACHE[key] = (plan, per_core, nc)
    plan, per_core, nc = _CACHE[key]

    in_maps = make_in_maps(x, W, a_src, a_dst, ep_w, ep_b, bias, per_core)
    res = run_bass_kernel_spmd(nc, in_maps, core_ids=list(range(NCORES)),
                               trace=False)
    out_full = np.empty((NPAD, OUT_F), dtype=np.float32)
    for c in range(NCORES):
        out_full[c * NPC: (c + 1) * NPC] = res.results[c]["out"]
    return out_full[:N]
